# revision 2
# baseline (speedup 1.0000x reference)
"""Trainium2 Bass kernel for nn_LAMME (conv3x3 + LAM temporal attention + ME gate).

Data-parallel over 8 NeuronCores: each core processes one clip of t=8 frames
(c=256, h=w=56).  Single fused kernel per core.

The 3x3 conv uses a 1-D Winograd F(2,3) transform along the ROW (height)
axis: input rows are split host-side into even/odd planes E/O of the 58x58
zero-padded frame; DVE computes 4 Winograd planes per row-tile r
  d0=E[r]-E[r+1], d1=O[r]+E[r+1], d2=E[r+1]-O[r], d3=O[r]-O[r+1]
and the PE contracts them against G-transformed weights (per dx shift), so
the PE streams 2/3 of the columns of a direct conv.  Two output rows come
back via e=m0+m1+m2, o=m1-m2-m3 (DVE adds on bf16 copies of PSUM).

The LAM softmax weights and ME sigmoid gates are a pure function of the
kernel INPUTS (pooled means of new_x derive analytically from window sums
of x), so they are precomputed on the host in _prep and shipped as a tiny
[128,4,2,8] constant.  The device kernel is then a clean two-stage pipeline:
conv(f) -> phase2(f-1) (temporal conv + gating + f32 output DMA) with no
mid-kernel dependency stalls.
"""
import sys
for p in ('/opt/trn_rl_repo',):
    if p not in sys.path:
        sys.path.insert(0, p)

import numpy as np
import ml_dtypes

import concourse.bacc as bacc
import concourse.mybir as mybir
import concourse.tile as tile
from concourse.bass_utils import run_bass_kernel_spmd

F32 = mybir.dt.float32
BF16 = mybir.dt.bfloat16
AF = mybir.ActivationFunctionType
OP = mybir.AluOpType

T = 8          # frames per clip (= clips per core after sharding)
NCORES = 8
HP = 58        # padded spatial width
PADSZ = HP * HP  # 3364
NT = 28        # winograd row-tiles per frame (56 out rows / 2)
NB = 4         # row-tile blocks per frame (7+7+7+7)
BLK = [(0, 7), (7, 7), (14, 7), (21, 7)]   # (tile0, ntiles) per block

_CACHE = {}


def _build():
    nc = bacc.Bacc("TRN2", target_bir_lowering=False, debug=False)

    # x layout per frame: [2ci_t, 128, 2eo, 29, 58] (even/odd padded rows)
    x_d = nc.dram_tensor("x", [T, 2, 128, PADSZ], BF16, kind="ExternalInput")
    gw_d = nc.dram_tensor("gw", [128, 6144], BF16, kind="ExternalInput")
    g_d = nc.dram_tensor("g", [128, 4, 2, T], F32, kind="ExternalInput")
    out_d = nc.dram_tensor("out", [T, 256, 3136], F32, kind="ExternalOutput")

    def cidx(co_t, ci_t, j, dx):
        return co_t * 24 + (ci_t * 4 + j) * 3 + dx

    with tile.TileContext(nc) as tc:
        with (
            tc.tile_pool(name="const", bufs=1) as cpool,
            tc.tile_pool(name="xp", bufs=4) as xpool,
            tc.tile_pool(name="dt", bufs=4) as dpool,
            tc.tile_pool(name="raw", bufs=4) as rawpool,
            tc.tile_pool(name="cp", bufs=2) as cppool,
            tc.tile_pool(name="work", bufs=2) as wpool,
            tc.tile_pool(name="fin", bufs=2) as fpool,
            tc.tile_pool(name="mpsum", bufs=2, space="PSUM") as mpsum,
        ):
            xin_t = {}
            # frame 0 split into two row-range tiles (rows 0-16 / 16-28 of
            # the E/O planes, row 14 duplicated) so the first transforms and
            # matmuls only wait on the first ~1MB of DMA
            xin0a = xpool.tile([128, 2, 2, 15, HP], BF16, tag="xin", name="xin")
            xin0b = xpool.tile([128, 2, 2, 15, HP], BF16, tag="xin", name="xin")
            gw_sb = cpool.tile([128, 48, 128], BF16)
            g_sb = cpool.tile([128, 4, 2, T], F32)
            for ci in range(2):
                xv0 = x_d.ap()[0, ci].rearrange("p (e r x) -> p e r x", e=2, x=HP)
                for eo in range(2):
                    nc.sync.dma_start(out=xin0a[:, ci, eo], in_=xv0[:, eo, 0:15, :])
            gwv = gw_d.ap().rearrange("p (c m) -> p c m", m=128)
            nc.sync.dma_start(out=gw_sb[:, 0:24], in_=gwv[:, 0:24])
            for ci in range(2):
                xv0 = x_d.ap()[0, ci].rearrange("p (e r x) -> p e r x", e=2, x=HP)
                for eo in range(2):
                    nc.sync.dma_start(out=xin0b[:, ci, eo], in_=xv0[:, eo, 14:29, :])
            nc.sync.dma_start(out=gw_sb[:, 24:48], in_=gwv[:, 24:48])
            nc.sync.dma_start(out=g_sb[:], in_=g_d.ap())

            def emit_xin_dma(f):
                xin = xpool.tile([128, 2, PADSZ], BF16, tag="xin", name="xin")
                xin_t[f] = xin
                nc.sync.dma_start(
                    out=xin[:], in_=x_d.ap()[f].rearrange("t p m -> p t m"))

            dt_t = {}

            def emit_transform(f, b):
                """winograd planes for row-tile block b of frame f"""
                r0, nt = BLK[b]
                if f == 0:
                    srv = xin0a if b < 2 else xin0b
                    rb = r0 if b < 2 else r0 - 14
                else:
                    srv = xin_t[f].rearrange("p c (e r x) -> p c e r x", e=2, x=HP)
                    rb = r0
                dt = dpool.tile([128, 2, 4, 7, HP], BF16, tag="dt", name="dt")
                dt_t[(f, b)] = dt
                E0 = srv[:, :, 0, rb:rb + nt, :]
                E1 = srv[:, :, 0, rb + 1:rb + nt + 1, :]
                O0 = srv[:, :, 1, rb:rb + nt, :]
                O1 = srv[:, :, 1, rb + 1:rb + nt + 1, :]
                nc.vector.tensor_sub(out=dt[:, :, 0, 0:nt], in0=E0, in1=E1)
                nc.vector.tensor_add(out=dt[:, :, 1, 0:nt], in0=O0, in1=E1)
                nc.vector.tensor_sub(out=dt[:, :, 2, 0:nt], in0=E1, in1=O0)
                nc.vector.tensor_sub(out=dt[:, :, 3, 0:nt], in0=O0, in1=O1)

            raw_tiles = {}

            def emit_conv_pair(f, co_t, pair):
                """blocks 2*pair, 2*pair+1: matmuls + PSUM->SBUF copy per
                block, then one merged inverse over the pair"""
                raw = raw_tiles[f]
                cp = cppool.tile([128, 4, 784], BF16, tag="cp", name="cp")
                off = 0
                for b in (2 * pair, 2 * pair + 1):
                    r0, nt = BLK[b]
                    W = nt * 56
                    dt = dt_t[(f, b)]
                    m = mpsum.tile([128, 4, 512], F32, tag="m", name="m")
                    for j in range(4):
                        idx = 0
                        for ci_t in range(2):
                            for dx in range(3):
                                nc.tensor.matmul(
                                    m[:, j, 0:W],
                                    gw_sb[:, cidx(co_t, ci_t, j, dx)],
                                    dt[:, ci_t, j, 0:nt, dx:dx + 56],
                                    start=(idx == 0), stop=(idx == 5))
                                idx += 1
                    nc.scalar.activation(
                        out=cp[:, :, off:off + W], in_=m[:, :, 0:W], func=AF.Copy)
                    off += W
                r0, _ = BLK[2 * pair]
                ntp = (BLK[2 * pair][1] + BLK[2 * pair + 1][1])
                WP = ntp * 56
                re = raw[:, co_t, 0, r0:r0 + ntp, :]
                ro = raw[:, co_t, 1, r0:r0 + ntp, :]
                nc.vector.tensor_add(out=re, in0=cp[:, 0, 0:WP], in1=cp[:, 1, 0:WP])
                nc.vector.tensor_add(out=re, in0=re, in1=cp[:, 2, 0:WP])
                nc.vector.tensor_sub(out=ro, in0=cp[:, 1, 0:WP], in1=cp[:, 2, 0:WP])
                nc.vector.tensor_sub(out=ro, in0=ro, in1=cp[:, 3, 0:WP])

            # ---------------- phase 2 (per frame, per row-tranche) --------
            def emit_phase2(f, trange):
                tr0, trn = trange
                for co_t in range(2):
                    fin = fpool.tile([128, 2 * trn, 56], F32, tag="fin", name="fin")
                    for eo in range(2):

                        def o(ff):
                            return raw_tiles[ff][:, co_t, eo, tr0:tr0 + trn, :]
                        fv = fin[:, eo::2, :]
                        A = wpool.tile([128, trn, 56], BF16, tag="A", name="A")
                        if f == 0 or f == T - 1:
                            fa, ka, fb, kb = (0, 1, 1, 2) if f == 0 else (T - 2, 0, T - 1, 1)
                            nc.scalar.activation(
                                out=A[:], in_=o(fa), func=AF.Identity,
                                scale=g_sb[:, ka, co_t, f:f + 1],
                                bias=g_sb[:, 3, co_t, f:f + 1])
                            nc.vector.scalar_tensor_tensor(
                                out=fv, in0=o(fb),
                                scalar=g_sb[:, kb, co_t, f:f + 1],
                                in1=A[:], op0=OP.mult, op1=OP.add)
                        else:
                            nc.scalar.activation(
                                out=A[:], in_=o(f - 1), func=AF.Identity,
                                scale=g_sb[:, 0, co_t, f:f + 1],
                                bias=g_sb[:, 3, co_t, f:f + 1])
                            Bp = wpool.tile([128, trn, 56], BF16, tag="Bp",
                                            name="Bp")
                            nc.scalar.mul(Bp[:], o(f), g_sb[:, 1, co_t, f:f + 1])
                            nc.gpsimd.tensor_add(out=A[:], in0=A[:], in1=Bp[:])
                            nc.vector.scalar_tensor_tensor(
                                out=fv, in0=o(f + 1),
                                scalar=g_sb[:, 2, co_t, f:f + 1],
                                in1=A[:], op0=OP.mult, op1=OP.add)
                    nc.sync.dma_start(
                        out=out_d.ap()[f, co_t * 128:(co_t + 1) * 128,
                                       tr0 * 112:(tr0 + trn) * 112],
                        in_=fin[:])

            # ---------------- schedule ------------------------------------
            # Per-engine FIFO order: transforms one frame ahead of conv,
            # phase2 lagging half a frame (tranche (0,14) after pair0 since
            # pair0 writes raw rows 0:14, tranche (14,14) after pair1).
            for f in (1, 2, 3):
                emit_xin_dma(f)
            for b in range(NB):
                emit_transform(0, b)
            for f in range(T):
                raw_tiles[f] = rawpool.tile([128, 2, 2, NT, 56], BF16,
                                            tag="raw", name="raw")
                for pair in range(2):
                    for co_t in range(2):
                        emit_conv_pair(f, co_t, pair)
                    if f < T - 1:
                        emit_transform(f + 1, 2 * pair)
                        emit_transform(f + 1, 2 * pair + 1)
                    tr = (0, 14) if pair == 0 else (14, 14)
                    if f >= 1:
                        emit_phase2(f - 1, tr)
                    if f == T - 1:
                        emit_phase2(f, tr)
                if 0 <= f <= 3:
                    emit_xin_dma(f + 4)

    nc.compile()
    return nc


def _host_gates(x, net_w, net_b, lam_w, lam_b, mlp_w1, bn_g, bn_b, bn_m,
                bn_v, mlp_w2, me_w):
    """LAM softmax weights + ME sigmoid gates, computed exactly as the
    reference does but from analytic window sums of x (the pooled means of
    new_x depend only on the inputs).  Returns [n, 128, 4, 2, T] f32 with
    which-axis = (g0, g1, g2, goffs)."""
    n, t, c = NCORES, T, 256
    xs = x.reshape(n, t, c, 56, 56)
    RS = xs.sum(axis=4)          # (n,t,c,56) per-row sums
    CS = xs.sum(axis=3)          # (n,t,c,56) per-col sums
    tot = RS.sum(axis=3)         # (n,t,c)
    # 3x3 window sums of the zero-padded frame: window (dy,dx) covers x rows
    # (dy-1..dy+54) clipped, i.e. drops row 55 (dy=0) or row 0 (dy=2); same
    # for cols; re-add the doubly-dropped corner.
    rdrop = [RS[..., 55], None, RS[..., 0]]
    cdrop = [CS[..., 55], None, CS[..., 0]]
    corner = {(0, 0): xs[..., 55, 55], (0, 2): xs[..., 55, 0],
              (2, 0): xs[..., 0, 55], (2, 2): xs[..., 0, 0]}
    S = np.empty((n, t, c, 3, 3), np.float32)
    for dy in range(3):
        for dx in range(3):
            v = tot.copy()
            if rdrop[dy] is not None:
                v -= rdrop[dy]
            if cdrop[dx] is not None:
                v -= cdrop[dx]
            if (dy, dx) in corner:
                v += corner[(dy, dx)]
            S[:, :, :, dy, dx] = v
    pooled_sum = S.reshape(n * t, c * 9) @ net_w.reshape(c, c * 9).T
    pooled_sum = pooled_sum.reshape(n, t, c)
    x_g = pooled_sum.mean(axis=1) / 3136.0 + net_b       # (n, c)
    x_g = x_g @ lam_w.T + lam_b
    bxg = net_b + x_g                                    # (n, c)
    pooled = pooled_sum.transpose(0, 2, 1) / 3136.0 + bxg[:, :, None]  # (n,c,t)
    hdn = pooled.reshape(n * c, t) @ mlp_w1.T
    scale = bn_g / np.sqrt(bn_v + 1e-5)
    hdn = (hdn - bn_m) * scale + bn_b
    hdn = np.maximum(hdn, 0.0)
    logits = hdn @ mlp_w2.T
    logits -= logits.max(axis=1, keepdims=True)
    e = np.exp(logits)
    wgt = (e / e.sum(axis=1, keepdims=True)).reshape(n, c, 3)
    # m[c,f] = mean_hw(lam_out) = temporal conv of pooled with wgt
    m = wgt[:, :, 1:2] * pooled
    m[:, :, 1:] += wgt[:, :, 0:1] * pooled[:, :, :-1]
    m[:, :, :-1] += wgt[:, :, 2:3] * pooled[:, :, 1:]
    y = np.zeros_like(m)
    y[:, :, :-1] = m[:, :, 1:] - m[:, :, :-1]
    yc = me_w[1] * y
    yc[:, 1:, :] += me_w[0] * y[:, :-1, :]
    yc[:, :-1, :] += me_w[2] * y[:, 1:, :]
    gate = 1.0 / (1.0 + np.exp(-yc))                     # (n, c, t)
    g0 = gate * wgt[:, :, 0:1]
    g1 = gate * wgt[:, :, 1:2]
    g2 = gate * wgt[:, :, 2:3]
    goffs = gate * bxg[:, :, None]
    goffs[:, :, 0] *= (wgt[:, :, 1] + wgt[:, :, 2])
    goffs[:, :, T - 1] *= (wgt[:, :, 0] + wgt[:, :, 1])
    arr = np.stack([g0, g1, g2, goffs], axis=1)          # (n, 4, c, t)
    arr = arr.reshape(n, 4, 2, 128, t).transpose(0, 3, 1, 2, 4)
    return np.ascontiguousarray(arr.astype(np.float32))


def _prep(inputs):
    x = np.asarray(inputs["x"], np.float32)          # (64,256,56,56)
    net_w = np.asarray(inputs["net_w"], np.float32)  # (256,256,3,3)
    net_b = np.asarray(inputs["net_b"], np.float32)
    lam_w = np.asarray(inputs["lam_w"], np.float32)
    lam_b = np.asarray(inputs["lam_b"], np.float32)
    mlp_w1 = np.asarray(inputs["mlp_w1"], np.float32)  # (16,8)
    mlp_w2 = np.asarray(inputs["mlp_w2"], np.float32)  # (3,16)
    bn_g = np.asarray(inputs["bn_gamma"], np.float32)
    bn_b = np.asarray(inputs["bn_beta"], np.float32)
    bn_m = np.asarray(inputs["bn_mean"], np.float32)
    bn_v = np.asarray(inputs["bn_var"], np.float32)
    me_w = np.asarray(inputs["me_w"], np.float32)

    bf = ml_dtypes.bfloat16
    xs = x.reshape(NCORES, T, 2, 128, 56, 56)
    xpad = np.zeros((NCORES, T, 2, 128, HP, HP), dtype=bf)
    xpad[:, :, :, :, 1:57, 1:57] = xs.astype(bf)
    # even/odd row planes: [..., 2, 29, 58]
    xeo = np.stack([xpad[:, :, :, :, 0::2, :], xpad[:, :, :, :, 1::2, :]], axis=4)
    xeo = np.ascontiguousarray(xeo.reshape(NCORES, T, 2, 128, PADSZ))

    # G-transformed (over dy) weights; flat [128, 48*128] so the DMA is one
    # contiguous 12.3KB run per partition.  Chunk index:
    # c = co_t*24 + (ci_t*4 + j)*3 + dx, chunk layout [p=ci128, m=co128].
    G = np.array([[1, 0, 0], [.5, .5, .5], [.5, -.5, .5], [0, 0, 1]], np.float32)
    gw_full = np.einsum('jy,oiyx->oijx', G, net_w)       # (256,256,4,3)
    arr = gw_full.reshape(2, 128, 2, 128, 4, 3).transpose(0, 2, 4, 5, 3, 1)
    gw = arr.reshape(48, 128, 128).transpose(1, 0, 2).reshape(128, 6144)
    gw = np.ascontiguousarray(gw.astype(bf))

    gates = _host_gates(x, net_w, net_b, lam_w, lam_b, mlp_w1, bn_g, bn_b,
                        bn_m, bn_v, mlp_w2, me_w)

    in_maps = [dict(x=xeo[i], gw=gw, g=gates[i]) for i in range(NCORES)]
    return in_maps


def kernel(**inputs):
    in_maps = _prep(inputs)
    nc = _CACHE.get('nc')
    if nc is None:
        nc = _build()
        _CACHE['nc'] = nc
    res = run_bass_kernel_spmd(nc, in_maps, core_ids=list(range(NCORES)))
    out = np.stack([res.results[i]["out"] for i in range(NCORES)])  # (8,8,256,3136)
    return np.ascontiguousarray(out.reshape(64, 256, 56, 56))


# revision 6
# speedup vs baseline: 1.1324x; 1.1324x over previous
"""Trainium2 Bass kernel for nn_LAMME (conv3x3 + LAM temporal attention + ME gate).

Data-parallel over 8 NeuronCores: each core processes one clip of t=8 frames
(c=256, h=w=56).  Single fused kernel per core.

The 3x3 conv uses a 1-D Winograd F(2,3) transform along the ROW (height)
axis: input rows are split host-side into even/odd planes E/O of the 58x58
zero-padded frame; DVE computes 4 Winograd planes per row-tile r
  d0=E[r]-E[r+1], d1=O[r]+E[r+1], d2=E[r+1]-O[r], d3=O[r]-O[r+1]
and the PE contracts them against G-transformed weights (per dx shift), so
the PE streams 2/3 of the columns of a direct conv.  Two output rows come
back via e=m0+m1+m2, o=m1-m2-m3 (DVE adds on bf16 copies of PSUM).

The LAM softmax weights and ME sigmoid gates are a pure function of the
kernel INPUTS (pooled means of new_x derive analytically from window sums
of x), so they are precomputed on the host in _prep and shipped as a tiny
[128,4,2,8] constant.  The device kernel is then a clean two-stage pipeline:
conv(f) -> phase2(f-1) (temporal conv + gating + f32 output DMA) with no
mid-kernel dependency stalls.
"""
import sys
for p in ('/opt/trn_rl_repo',):
    if p not in sys.path:
        sys.path.insert(0, p)

import numpy as np
import ml_dtypes

import concourse.bacc as bacc
import concourse.mybir as mybir
import concourse.tile as tile
from concourse.bass_utils import run_bass_kernel_spmd

F32 = mybir.dt.float32
BF16 = mybir.dt.bfloat16
AF = mybir.ActivationFunctionType
OP = mybir.AluOpType

T = 8          # frames per clip (= clips per core after sharding)
NCORES = 8
HP = 58        # padded spatial width
PADSZ = HP * HP  # 3364
NT = 28        # winograd row-tiles per frame (56 out rows / 2)
NB = 4         # row-tile blocks per frame (7+7+7+7)
BLK = [(0, 7), (7, 7), (14, 7), (21, 7)]   # (tile0, ntiles) per block

_CACHE = {}


def _build():
    nc = bacc.Bacc("TRN2", target_bir_lowering=False, debug=False)

    # x layout per frame: [2ci_t, 128, 2eo, 29, 58] (even/odd padded rows)
    x_d = nc.dram_tensor("x", [T, 2, 128, PADSZ], BF16, kind="ExternalInput")
    gw_d = nc.dram_tensor("gw", [128, 6144], BF16, kind="ExternalInput")
    g_d = nc.dram_tensor("g", [128, 4, 2, T], F32, kind="ExternalInput")
    out_d = nc.dram_tensor("out", [T, 256, 3136], F32, kind="ExternalOutput")

    def cidx(co_t, ci_t, j, dx):
        return co_t * 24 + (ci_t * 4 + j) * 3 + dx

    with tile.TileContext(nc) as tc:
        with (
            tc.tile_pool(name="const", bufs=1) as cpool,
            tc.tile_pool(name="xp", bufs=4) as xpool,
            tc.tile_pool(name="dt", bufs=4) as dpool,
            tc.tile_pool(name="raw", bufs=4) as rawpool,
            tc.tile_pool(name="cp", bufs=2) as cppool,
            tc.tile_pool(name="work", bufs=4) as wpool,
            tc.tile_pool(name="fin", bufs=2) as fpool,
            tc.tile_pool(name="mpsum", bufs=2, space="PSUM") as mpsum,
        ):
            xin_t = {}
            # frame 0 split into two row-range tiles (rows 0-16 / 16-28 of
            # the E/O planes, row 14 duplicated) so the first transforms and
            # matmuls only wait on the first ~1MB of DMA
            xin0a = xpool.tile([128, 2, 2, 15, HP], BF16, tag="xin", name="xin")
            xin0b = xpool.tile([128, 2, 2, 15, HP], BF16, tag="xin", name="xin")
            gw_sb = cpool.tile([128, 48, 128], BF16)
            g_sb = cpool.tile([128, 4, 2, T], F32)
            gwv = gw_d.ap().rearrange("p (c m) -> p c m", m=128)
            for ci in range(2):
                xv0 = x_d.ap()[0, ci].rearrange("p (e r x) -> p e r x", e=2, x=HP)
                nc.sync.dma_start(out=xin0a[:, ci], in_=xv0[:, :, 0:15, :])
            nc.sync.dma_start(out=gw_sb[:, 0:24], in_=gwv[:, 0:24])
            for ci in range(2):
                xv0 = x_d.ap()[0, ci].rearrange("p (e r x) -> p e r x", e=2, x=HP)
                nc.sync.dma_start(out=xin0b[:, ci], in_=xv0[:, :, 14:29, :])
            nc.sync.dma_start(out=gw_sb[:, 24:48], in_=gwv[:, 24:48])
            nc.sync.dma_start(out=g_sb[:], in_=g_d.ap())

            def emit_xin_dma(f):
                # frame loads dispatch from the (idle) GpSimd queue so the
                # Sync queue only carries weights + output stores
                xin = xpool.tile([128, 2, PADSZ], BF16, tag="xin", name="xin")
                xin_t[f] = xin
                nc.gpsimd.dma_start(
                    out=xin[:], in_=x_d.ap()[f].rearrange("t p m -> p t m"))

            dt_t = {}

            def emit_transform(f, b):
                """winograd planes for row-tile block b of frame f"""
                r0, nt = BLK[b]
                if f == 0:
                    srv = xin0a if b < 2 else xin0b
                    rb = r0 if b < 2 else r0 - 14
                else:
                    srv = xin_t[f].rearrange("p c (e r x) -> p c e r x", e=2, x=HP)
                    rb = r0
                dt = dpool.tile([128, 2, 4, 7, HP], BF16, tag="dt", name="dt")
                dt_t[(f, b)] = dt
                E0 = srv[:, :, 0, rb:rb + nt, :]
                E1 = srv[:, :, 0, rb + 1:rb + nt + 1, :]
                O0 = srv[:, :, 1, rb:rb + nt, :]
                O1 = srv[:, :, 1, rb + 1:rb + nt + 1, :]
                nc.vector.tensor_sub(out=dt[:, :, 0, 0:nt], in0=E0, in1=E1)
                nc.vector.tensor_add(out=dt[:, :, 1, 0:nt], in0=O0, in1=E1)
                nc.vector.tensor_sub(out=dt[:, :, 2, 0:nt], in0=E1, in1=O0)
                nc.vector.tensor_sub(out=dt[:, :, 3, 0:nt], in0=O0, in1=O1)

            raw_tiles = {}

            def emit_conv_pair(f, co_t, pair):
                """blocks 2*pair, 2*pair+1: matmuls + PSUM->SBUF copy per
                block, then one merged inverse over the pair"""
                raw = raw_tiles[f]
                cp = cppool.tile([128, 4, 784], BF16, tag="cp", name="cp")
                off = 0
                for b in (2 * pair, 2 * pair + 1):
                    r0, nt = BLK[b]
                    W = nt * 56
                    dt = dt_t[(f, b)]
                    m = mpsum.tile([128, 4, 512], F32, tag="m", name="m")
                    for j in range(4):
                        idx = 0
                        for ci_t in range(2):
                            for dx in range(3):
                                nc.tensor.matmul(
                                    m[:, j, 0:W],
                                    gw_sb[:, cidx(co_t, ci_t, j, dx)],
                                    dt[:, ci_t, j, 0:nt, dx:dx + 56],
                                    start=(idx == 0), stop=(idx == 5))
                                idx += 1
                    nc.scalar.activation(
                        out=cp[:, :, off:off + W], in_=m[:, :, 0:W], func=AF.Copy)
                    off += W
                r0, _ = BLK[2 * pair]
                ntp = (BLK[2 * pair][1] + BLK[2 * pair + 1][1])
                WP = ntp * 56
                re = raw[:, co_t, 0, r0:r0 + ntp, :]
                ro = raw[:, co_t, 1, r0:r0 + ntp, :]
                nc.vector.tensor_add(out=re, in0=cp[:, 0, 0:WP], in1=cp[:, 1, 0:WP])
                nc.vector.tensor_add(out=re, in0=re, in1=cp[:, 2, 0:WP])
                nc.vector.tensor_sub(out=ro, in0=cp[:, 1, 0:WP], in1=cp[:, 2, 0:WP])
                nc.vector.tensor_sub(out=ro, in0=ro, in1=cp[:, 3, 0:WP])

            # ---------------- phase 2 (per frame, per row-tranche) --------
            def emit_phase2(f, trange):
                tr0, trn = trange
                for co_t in range(2):
                    fin = fpool.tile([128, 2 * trn, 56], F32, tag="fin", name="fin")
                    for eo in range(2):

                        def o(ff):
                            return raw_tiles[ff][:, co_t, eo, tr0:tr0 + trn, :]
                        fv = fin[:, eo::2, :]
                        A = wpool.tile([128, trn, 56], BF16, tag="A", name="A")
                        if f == 0 or f == T - 1:
                            fa, ka, fb, kb = (0, 1, 1, 2) if f == 0 else (T - 2, 0, T - 1, 1)
                            nc.scalar.activation(
                                out=A[:], in_=o(fa), func=AF.Identity,
                                scale=g_sb[:, ka, co_t, f:f + 1],
                                bias=g_sb[:, 3, co_t, f:f + 1])
                            nc.vector.scalar_tensor_tensor(
                                out=fv, in0=o(fb),
                                scalar=g_sb[:, kb, co_t, f:f + 1],
                                in1=A[:], op0=OP.mult, op1=OP.add)
                        else:
                            nc.scalar.activation(
                                out=A[:], in_=o(f - 1), func=AF.Identity,
                                scale=g_sb[:, 0, co_t, f:f + 1],
                                bias=g_sb[:, 3, co_t, f:f + 1])
                            nc.vector.scalar_tensor_tensor(
                                out=A[:], in0=o(f),
                                scalar=g_sb[:, 1, co_t, f:f + 1],
                                in1=A[:], op0=OP.mult, op1=OP.add)
                            nc.vector.scalar_tensor_tensor(
                                out=fv, in0=o(f + 1),
                                scalar=g_sb[:, 2, co_t, f:f + 1],
                                in1=A[:], op0=OP.mult, op1=OP.add)
                    nc.sync.dma_start(
                        out=out_d.ap()[f, co_t * 128:(co_t + 1) * 128,
                                       tr0 * 112:(tr0 + trn) * 112],
                        in_=fin[:])

            # ---------------- schedule ------------------------------------
            # Per-engine FIFO order: transforms one frame ahead of conv,
            # phase2 lagging half a frame (tranche (0,14) after pair0 since
            # pair0 writes raw rows 0:14, tranche (14,14) after pair1).
            for f in (1, 2, 3):
                emit_xin_dma(f)
            for b in range(NB):
                emit_transform(0, b)
            for f in range(T):
                raw_tiles[f] = rawpool.tile([128, 2, 2, NT, 56], BF16,
                                            tag="raw", name="raw")
                for pair in range(2):
                    for co_t in range(2):
                        emit_conv_pair(f, co_t, pair)
                    if f < T - 1:
                        emit_transform(f + 1, 2 * pair)
                        emit_transform(f + 1, 2 * pair + 1)
                    tr = (0, 14) if pair == 0 else (14, 14)
                    if f >= 1:
                        emit_phase2(f - 1, tr)
                    if f == T - 1:
                        emit_phase2(f, tr)
                if 0 <= f <= 3:
                    emit_xin_dma(f + 4)

    nc.compile()
    return nc


def _host_gates(x, net_w, net_b, lam_w, lam_b, mlp_w1, bn_g, bn_b, bn_m,
                bn_v, mlp_w2, me_w):
    """LAM softmax weights + ME sigmoid gates, computed exactly as the
    reference does but from analytic window sums of x (the pooled means of
    new_x depend only on the inputs).  Returns [n, 128, 4, 2, T] f32 with
    which-axis = (g0, g1, g2, goffs)."""
    n, t, c = NCORES, T, 256
    xs = x.reshape(n, t, c, 56, 56)
    RS = xs.sum(axis=4)          # (n,t,c,56) per-row sums
    CS = xs.sum(axis=3)          # (n,t,c,56) per-col sums
    tot = RS.sum(axis=3)         # (n,t,c)
    # 3x3 window sums of the zero-padded frame: window (dy,dx) covers x rows
    # (dy-1..dy+54) clipped, i.e. drops row 55 (dy=0) or row 0 (dy=2); same
    # for cols; re-add the doubly-dropped corner.
    rdrop = [RS[..., 55], None, RS[..., 0]]
    cdrop = [CS[..., 55], None, CS[..., 0]]
    corner = {(0, 0): xs[..., 55, 55], (0, 2): xs[..., 55, 0],
              (2, 0): xs[..., 0, 55], (2, 2): xs[..., 0, 0]}
    S = np.empty((n, t, c, 3, 3), np.float32)
    for dy in range(3):
        for dx in range(3):
            v = tot.copy()
            if rdrop[dy] is not None:
                v -= rdrop[dy]
            if cdrop[dx] is not None:
                v -= cdrop[dx]
            if (dy, dx) in corner:
                v += corner[(dy, dx)]
            S[:, :, :, dy, dx] = v
    pooled_sum = S.reshape(n * t, c * 9) @ net_w.reshape(c, c * 9).T
    pooled_sum = pooled_sum.reshape(n, t, c)
    x_g = pooled_sum.mean(axis=1) / 3136.0 + net_b       # (n, c)
    x_g = x_g @ lam_w.T + lam_b
    bxg = net_b + x_g                                    # (n, c)
    pooled = pooled_sum.transpose(0, 2, 1) / 3136.0 + bxg[:, :, None]  # (n,c,t)
    hdn = pooled.reshape(n * c, t) @ mlp_w1.T
    scale = bn_g / np.sqrt(bn_v + 1e-5)
    hdn = (hdn - bn_m) * scale + bn_b
    hdn = np.maximum(hdn, 0.0)
    logits = hdn @ mlp_w2.T
    logits -= logits.max(axis=1, keepdims=True)
    e = np.exp(logits)
    wgt = (e / e.sum(axis=1, keepdims=True)).reshape(n, c, 3)
    # m[c,f] = mean_hw(lam_out) = temporal conv of pooled with wgt
    m = wgt[:, :, 1:2] * pooled
    m[:, :, 1:] += wgt[:, :, 0:1] * pooled[:, :, :-1]
    m[:, :, :-1] += wgt[:, :, 2:3] * pooled[:, :, 1:]
    y = np.zeros_like(m)
    y[:, :, :-1] = m[:, :, 1:] - m[:, :, :-1]
    yc = me_w[1] * y
    yc[:, 1:, :] += me_w[0] * y[:, :-1, :]
    yc[:, :-1, :] += me_w[2] * y[:, 1:, :]
    gate = 1.0 / (1.0 + np.exp(-yc))                     # (n, c, t)
    g0 = gate * wgt[:, :, 0:1]
    g1 = gate * wgt[:, :, 1:2]
    g2 = gate * wgt[:, :, 2:3]
    goffs = gate * bxg[:, :, None]
    goffs[:, :, 0] *= (wgt[:, :, 1] + wgt[:, :, 2])
    goffs[:, :, T - 1] *= (wgt[:, :, 0] + wgt[:, :, 1])
    arr = np.stack([g0, g1, g2, goffs], axis=1)          # (n, 4, c, t)
    arr = arr.reshape(n, 4, 2, 128, t).transpose(0, 3, 1, 2, 4)
    return np.ascontiguousarray(arr.astype(np.float32))


def _prep(inputs):
    x = np.asarray(inputs["x"], np.float32)          # (64,256,56,56)
    net_w = np.asarray(inputs["net_w"], np.float32)  # (256,256,3,3)
    net_b = np.asarray(inputs["net_b"], np.float32)
    lam_w = np.asarray(inputs["lam_w"], np.float32)
    lam_b = np.asarray(inputs["lam_b"], np.float32)
    mlp_w1 = np.asarray(inputs["mlp_w1"], np.float32)  # (16,8)
    mlp_w2 = np.asarray(inputs["mlp_w2"], np.float32)  # (3,16)
    bn_g = np.asarray(inputs["bn_gamma"], np.float32)
    bn_b = np.asarray(inputs["bn_beta"], np.float32)
    bn_m = np.asarray(inputs["bn_mean"], np.float32)
    bn_v = np.asarray(inputs["bn_var"], np.float32)
    me_w = np.asarray(inputs["me_w"], np.float32)

    bf = ml_dtypes.bfloat16
    xs = x.reshape(NCORES, T, 2, 128, 56, 56)
    xpad = np.zeros((NCORES, T, 2, 128, HP, HP), dtype=bf)
    xpad[:, :, :, :, 1:57, 1:57] = xs.astype(bf)
    # even/odd row planes: [..., 2, 29, 58]
    xeo = np.stack([xpad[:, :, :, :, 0::2, :], xpad[:, :, :, :, 1::2, :]], axis=4)
    xeo = np.ascontiguousarray(xeo.reshape(NCORES, T, 2, 128, PADSZ))

    # G-transformed (over dy) weights; flat [128, 48*128] so the DMA is one
    # contiguous 12.3KB run per partition.  Chunk index:
    # c = co_t*24 + (ci_t*4 + j)*3 + dx, chunk layout [p=ci128, m=co128].
    G = np.array([[1, 0, 0], [.5, .5, .5], [.5, -.5, .5], [0, 0, 1]], np.float32)
    gw_full = np.einsum('jy,oiyx->oijx', G, net_w)       # (256,256,4,3)
    arr = gw_full.reshape(2, 128, 2, 128, 4, 3).transpose(0, 2, 4, 5, 3, 1)
    gw = arr.reshape(48, 128, 128).transpose(1, 0, 2).reshape(128, 6144)
    gw = np.ascontiguousarray(gw.astype(bf))

    gates = _host_gates(x, net_w, net_b, lam_w, lam_b, mlp_w1, bn_g, bn_b,
                        bn_m, bn_v, mlp_w2, me_w)

    in_maps = [dict(x=xeo[i], gw=gw, g=gates[i]) for i in range(NCORES)]
    return in_maps


def kernel(**inputs):
    in_maps = _prep(inputs)
    nc = _CACHE.get('nc')
    if nc is None:
        nc = _build()
        _CACHE['nc'] = nc
    res = run_bass_kernel_spmd(nc, in_maps, core_ids=list(range(NCORES)))
    out = np.stack([res.results[i]["out"] for i in range(NCORES)])  # (8,8,256,3136)
    return np.ascontiguousarray(out.reshape(64, 256, 56, 56))


# revision 11
# speedup vs baseline: 1.1940x; 1.0544x over previous
"""Trainium2 Bass kernel for nn_LAMME (conv3x3 + LAM temporal attention + ME gate).

Data-parallel over 8 NeuronCores: each core processes one clip of t=8 frames
(c=256, h=w=56).  Single fused kernel per core.

The 3x3 conv uses a 1-D Winograd F(2,3) transform along the ROW (height)
axis: input rows are split host-side into even/odd planes E/O of the 58x58
zero-padded frame; DVE computes 4 Winograd planes per row-tile r
  d0=E[r]-E[r+1], d1=O[r]+E[r+1], d2=E[r+1]-O[r], d3=O[r]-O[r+1]
and the PE contracts them against G-transformed weights (per dx shift), so
the PE streams 2/3 of the columns of a direct conv.  Two output rows come
back via e=m0+m1+m2, o=m1-m2-m3 (DVE adds on bf16 copies of PSUM).

The LAM softmax weights and ME sigmoid gates are a pure function of the
kernel INPUTS (pooled means of new_x derive analytically from window sums
of x), so they are precomputed on the host in _prep and shipped as a tiny
[128,4,2,8] constant.  The device kernel is then a clean two-stage pipeline:
conv(f) -> phase2(f-1) (temporal conv + gating + f32 output DMA) with no
mid-kernel dependency stalls.
"""
import sys
for p in ('/opt/trn_rl_repo',):
    if p not in sys.path:
        sys.path.insert(0, p)

import numpy as np
import ml_dtypes

import concourse.bacc as bacc
import concourse.mybir as mybir
import concourse.tile as tile
from concourse.bass_utils import run_bass_kernel_spmd

F32 = mybir.dt.float32
BF16 = mybir.dt.bfloat16
AF = mybir.ActivationFunctionType
OP = mybir.AluOpType

T = 8          # frames per clip (= clips per core after sharding)
NCORES = 8
HP = 58        # padded spatial width
PADSZ = HP * HP  # 3364
NT = 28        # winograd row-tiles per frame (56 out rows / 2)
NB = 4         # row-tile blocks per frame (7+7+7+7)
BLK = [(0, 7), (7, 7), (14, 7), (21, 7)]   # (tile0, ntiles) per block

_CACHE = {}


def _build():
    nc = bacc.Bacc("TRN2", target_bir_lowering=False, debug=False)

    # x layout per frame: [2ci_t, 128, 2eo, 29, 58] (even/odd padded rows)
    x_d = nc.dram_tensor("x", [T, 2, 128, PADSZ], BF16, kind="ExternalInput")
    gw_d = nc.dram_tensor("gw", [128, 6144], BF16, kind="ExternalInput")
    g_d = nc.dram_tensor("g", [128, 4, 2, T], F32, kind="ExternalInput")
    out_d = nc.dram_tensor("out", [T, 256, 3136], F32, kind="ExternalOutput")

    def cidx(co_t, ci_t, j, dx):
        # j-major within a co half so the first DMA split ([0:6], one j
        # group) unblocks the first matmuls
        return co_t * 24 + j * 6 + ci_t * 3 + dx

    with tile.TileContext(nc) as tc:
        with (
            tc.tile_pool(name="const", bufs=1) as cpool,
            tc.tile_pool(name="xp", bufs=4) as xpool,
            tc.tile_pool(name="dt", bufs=4) as dpool,
            tc.tile_pool(name="raw", bufs=4) as rawpool,
            tc.tile_pool(name="cp", bufs=2) as cppool,
            tc.tile_pool(name="work", bufs=4) as wpool,
            tc.tile_pool(name="fin", bufs=2) as fpool,
            tc.tile_pool(name="mpsum", bufs=2, space="PSUM") as mpsum,
        ):
            xin_t = {}
            # frame 0 split into two row-range tiles (rows 0-16 / 16-28 of
            # the E/O planes, row 14 duplicated) so the first transforms and
            # matmuls only wait on the first ~1MB of DMA
            xin0a = xpool.tile([128, 2, 2, 15, HP], BF16, tag="xin", name="xin")
            xin0b = xpool.tile([128, 2, 2, 15, HP], BF16, tag="xin", name="xin")
            gw_sb = cpool.tile([128, 48, 128], BF16)
            g_sb = cpool.tile([128, 4, 2, T], F32)
            gwv = gw_d.ap().rearrange("p (c m) -> p c m", m=128)
            for ci in range(2):
                xv0 = x_d.ap()[0, ci].rearrange("p (e r x) -> p e r x", e=2, x=HP)
                nc.sync.dma_start(out=xin0a[:, ci], in_=xv0[:, :, 0:15, :])
            # critical-path order: j0 chunks, rest of co0, co1, then frame 0b
            nc.sync.dma_start(out=gw_sb[:, 0:6], in_=gwv[:, 0:6])
            nc.sync.dma_start(out=gw_sb[:, 6:24], in_=gwv[:, 6:24])
            nc.sync.dma_start(out=gw_sb[:, 24:48], in_=gwv[:, 24:48])
            for ci in range(2):
                xv0 = x_d.ap()[0, ci].rearrange("p (e r x) -> p e r x", e=2, x=HP)
                nc.sync.dma_start(out=xin0b[:, ci], in_=xv0[:, :, 14:29, :])
            nc.sync.dma_start(out=g_sb[:], in_=g_d.ap())

            def emit_xin_dma(f):
                xin = xpool.tile([128, 2, PADSZ], BF16, tag="xin", name="xin")
                xin_t[f] = xin
                nc.sync.dma_start(
                    out=xin[:], in_=x_d.ap()[f].rearrange("t p m -> p t m"))

            dt_t = {}

            def emit_transform(f, b):
                """winograd planes for row-tile block b of frame f"""
                r0, nt = BLK[b]
                if f == 0:
                    srv = xin0a if b < 2 else xin0b
                    rb = r0 if b < 2 else r0 - 14
                else:
                    srv = xin_t[f].rearrange("p c (e r x) -> p c e r x", e=2, x=HP)
                    rb = r0
                dt = dpool.tile([128, 2, 4, 7, HP], BF16, tag="dt", name="dt")
                dt_t[(f, b)] = dt
                E0 = srv[:, :, 0, rb:rb + nt, :]
                E1 = srv[:, :, 0, rb + 1:rb + nt + 1, :]
                O0 = srv[:, :, 1, rb:rb + nt, :]
                O1 = srv[:, :, 1, rb + 1:rb + nt + 1, :]
                nc.vector.tensor_sub(out=dt[:, :, 0, 0:nt], in0=E0, in1=E1)
                nc.vector.tensor_add(out=dt[:, :, 1, 0:nt], in0=O0, in1=E1)
                nc.vector.tensor_sub(out=dt[:, :, 2, 0:nt], in0=E1, in1=O0)
                nc.vector.tensor_sub(out=dt[:, :, 3, 0:nt], in0=O0, in1=O1)

            raw_tiles = {}

            def emit_conv_pair(f, co_t, pair):
                """blocks 2*pair, 2*pair+1: matmuls + PSUM->SBUF copy per
                block, then one merged inverse over the pair"""
                raw = raw_tiles[f]
                cp = cppool.tile([128, 4, 784], BF16, tag="cp", name="cp")
                off = 0
                for b in (2 * pair, 2 * pair + 1):
                    r0, nt = BLK[b]
                    W = nt * 56
                    dt = dt_t[(f, b)]
                    m = mpsum.tile([128, 4, 512], F32, tag="m", name="m")
                    for j in range(4):
                        idx = 0
                        for ci_t in range(2):
                            for dx in range(3):
                                nc.tensor.matmul(
                                    m[:, j, 0:W],
                                    gw_sb[:, cidx(co_t, ci_t, j, dx)],
                                    dt[:, ci_t, j, 0:nt, dx:dx + 56],
                                    start=(idx == 0), stop=(idx == 5))
                                idx += 1
                    nc.scalar.activation(
                        out=cp[:, :, off:off + W], in_=m[:, :, 0:W], func=AF.Copy)
                    off += W
                r0, _ = BLK[2 * pair]
                ntp = (BLK[2 * pair][1] + BLK[2 * pair + 1][1])
                WP = ntp * 56
                re = raw[:, co_t, 0, r0:r0 + ntp, :]
                ro = raw[:, co_t, 1, r0:r0 + ntp, :]
                nc.vector.tensor_add(out=re, in0=cp[:, 0, 0:WP], in1=cp[:, 1, 0:WP])
                nc.vector.tensor_add(out=re, in0=re, in1=cp[:, 2, 0:WP])
                nc.vector.tensor_sub(out=ro, in0=cp[:, 1, 0:WP], in1=cp[:, 2, 0:WP])
                nc.vector.tensor_sub(out=ro, in0=ro, in1=cp[:, 3, 0:WP])

            # ---------------- phase 2 (per frame, per row-tranche) --------
            def emit_phase2(f, trange, cos=(0, 1)):
                tr0, trn = trange
                for co_t in cos:
                    fin = fpool.tile([128, 2 * trn, 56], F32, tag="fin", name="fin")
                    for eo in range(2):

                        def o(ff):
                            return raw_tiles[ff][:, co_t, eo, tr0:tr0 + trn, :]
                        fv = fin[:, eo::2, :]
                        A = wpool.tile([128, trn, 56], BF16, tag="A", name="A")
                        if f == 0 or f == T - 1:
                            fa, ka, fb, kb = (0, 1, 1, 2) if f == 0 else (T - 2, 0, T - 1, 1)
                            nc.scalar.activation(
                                out=A[:], in_=o(fa), func=AF.Identity,
                                scale=g_sb[:, ka, co_t, f:f + 1],
                                bias=g_sb[:, 3, co_t, f:f + 1])
                            nc.vector.scalar_tensor_tensor(
                                out=fv, in0=o(fb),
                                scalar=g_sb[:, kb, co_t, f:f + 1],
                                in1=A[:], op0=OP.mult, op1=OP.add)
                        else:
                            nc.scalar.activation(
                                out=A[:], in_=o(f - 1), func=AF.Identity,
                                scale=g_sb[:, 0, co_t, f:f + 1],
                                bias=g_sb[:, 3, co_t, f:f + 1])
                            nc.vector.scalar_tensor_tensor(
                                out=A[:], in0=o(f),
                                scalar=g_sb[:, 1, co_t, f:f + 1],
                                in1=A[:], op0=OP.mult, op1=OP.add)
                            nc.vector.scalar_tensor_tensor(
                                out=fv, in0=o(f + 1),
                                scalar=g_sb[:, 2, co_t, f:f + 1],
                                in1=A[:], op0=OP.mult, op1=OP.add)
                    nc.sync.dma_start(
                        out=out_d.ap()[f, co_t * 128:(co_t + 1) * 128,
                                       tr0 * 112:(tr0 + trn) * 112],
                        in_=fin[:])

            # ---------------- schedule ------------------------------------
            # Per-engine FIFO order: transforms one frame ahead of conv,
            # phase2 lagging half a frame (tranche (0,14) after pair0 since
            # pair0 writes raw rows 0:14, tranche (14,14) after pair1).
            for f in (1, 2, 3):
                emit_xin_dma(f)
            for b in range(NB):
                emit_transform(0, b)
            for f in range(T - 1):
                raw_tiles[f] = rawpool.tile([128, 2, 2, NT, 56], BF16,
                                            tag="raw", name="raw")
                for pair in range(2):
                    for co_t in range(2):
                        emit_conv_pair(f, co_t, pair)
                    emit_transform(f + 1, 2 * pair)
                    emit_transform(f + 1, 2 * pair + 1)
                    if f >= 1:
                        emit_phase2(f - 1, (0, 14) if pair == 0 else (14, 14))
                if 0 <= f <= 3:
                    emit_xin_dma(f + 4)
            # last frame: phase2 of rows 0:14 runs during pair1, and the
            # rows 14:28 phase2 of co0 runs while pair1/co1 matmuls stream
            f = T - 1
            raw_tiles[f] = rawpool.tile([128, 2, 2, NT, 56], BF16,
                                        tag="raw", name="raw")
            for co_t in range(2):
                emit_conv_pair(f, co_t, 0)
            emit_phase2(f - 1, (0, 14))
            emit_phase2(f, (0, 14))
            emit_conv_pair(f, 0, 1)
            emit_phase2(f - 1, (14, 14), cos=(0,))
            emit_phase2(f, (14, 14), cos=(0,))
            emit_conv_pair(f, 1, 1)
            emit_phase2(f - 1, (14, 14), cos=(1,))
            emit_phase2(f, (14, 14), cos=(1,))

    nc.compile()
    return nc


def _host_gates(x, net_w, net_b, lam_w, lam_b, mlp_w1, bn_g, bn_b, bn_m,
                bn_v, mlp_w2, me_w):
    """LAM softmax weights + ME sigmoid gates, computed exactly as the
    reference does but from analytic window sums of x (the pooled means of
    new_x depend only on the inputs).  Returns [n, 128, 4, 2, T] f32 with
    which-axis = (g0, g1, g2, goffs)."""
    n, t, c = NCORES, T, 256
    xs = x.reshape(n, t, c, 56, 56)
    RS = xs.sum(axis=4)          # (n,t,c,56) per-row sums
    CS = xs.sum(axis=3)          # (n,t,c,56) per-col sums
    tot = RS.sum(axis=3)         # (n,t,c)
    # 3x3 window sums of the zero-padded frame: window (dy,dx) covers x rows
    # (dy-1..dy+54) clipped, i.e. drops row 55 (dy=0) or row 0 (dy=2); same
    # for cols; re-add the doubly-dropped corner.
    rdrop = [RS[..., 55], None, RS[..., 0]]
    cdrop = [CS[..., 55], None, CS[..., 0]]
    corner = {(0, 0): xs[..., 55, 55], (0, 2): xs[..., 55, 0],
              (2, 0): xs[..., 0, 55], (2, 2): xs[..., 0, 0]}
    S = np.empty((n, t, c, 3, 3), np.float32)
    for dy in range(3):
        for dx in range(3):
            v = tot.copy()
            if rdrop[dy] is not None:
                v -= rdrop[dy]
            if cdrop[dx] is not None:
                v -= cdrop[dx]
            if (dy, dx) in corner:
                v += corner[(dy, dx)]
            S[:, :, :, dy, dx] = v
    pooled_sum = S.reshape(n * t, c * 9) @ net_w.reshape(c, c * 9).T
    pooled_sum = pooled_sum.reshape(n, t, c)
    x_g = pooled_sum.mean(axis=1) / 3136.0 + net_b       # (n, c)
    x_g = x_g @ lam_w.T + lam_b
    bxg = net_b + x_g                                    # (n, c)
    pooled = pooled_sum.transpose(0, 2, 1) / 3136.0 + bxg[:, :, None]  # (n,c,t)
    hdn = pooled.reshape(n * c, t) @ mlp_w1.T
    scale = bn_g / np.sqrt(bn_v + 1e-5)
    hdn = (hdn - bn_m) * scale + bn_b
    hdn = np.maximum(hdn, 0.0)
    logits = hdn @ mlp_w2.T
    logits -= logits.max(axis=1, keepdims=True)
    e = np.exp(logits)
    wgt = (e / e.sum(axis=1, keepdims=True)).reshape(n, c, 3)
    # m[c,f] = mean_hw(lam_out) = temporal conv of pooled with wgt
    m = wgt[:, :, 1:2] * pooled
    m[:, :, 1:] += wgt[:, :, 0:1] * pooled[:, :, :-1]
    m[:, :, :-1] += wgt[:, :, 2:3] * pooled[:, :, 1:]
    y = np.zeros_like(m)
    y[:, :, :-1] = m[:, :, 1:] - m[:, :, :-1]
    yc = me_w[1] * y
    yc[:, 1:, :] += me_w[0] * y[:, :-1, :]
    yc[:, :-1, :] += me_w[2] * y[:, 1:, :]
    gate = 1.0 / (1.0 + np.exp(-yc))                     # (n, c, t)
    g0 = gate * wgt[:, :, 0:1]
    g1 = gate * wgt[:, :, 1:2]
    g2 = gate * wgt[:, :, 2:3]
    goffs = gate * bxg[:, :, None]
    goffs[:, :, 0] *= (wgt[:, :, 1] + wgt[:, :, 2])
    goffs[:, :, T - 1] *= (wgt[:, :, 0] + wgt[:, :, 1])
    arr = np.stack([g0, g1, g2, goffs], axis=1)          # (n, 4, c, t)
    arr = arr.reshape(n, 4, 2, 128, t).transpose(0, 3, 1, 2, 4)
    return np.ascontiguousarray(arr.astype(np.float32))


def _prep(inputs):
    x = np.asarray(inputs["x"], np.float32)          # (64,256,56,56)
    net_w = np.asarray(inputs["net_w"], np.float32)  # (256,256,3,3)
    net_b = np.asarray(inputs["net_b"], np.float32)
    lam_w = np.asarray(inputs["lam_w"], np.float32)
    lam_b = np.asarray(inputs["lam_b"], np.float32)
    mlp_w1 = np.asarray(inputs["mlp_w1"], np.float32)  # (16,8)
    mlp_w2 = np.asarray(inputs["mlp_w2"], np.float32)  # (3,16)
    bn_g = np.asarray(inputs["bn_gamma"], np.float32)
    bn_b = np.asarray(inputs["bn_beta"], np.float32)
    bn_m = np.asarray(inputs["bn_mean"], np.float32)
    bn_v = np.asarray(inputs["bn_var"], np.float32)
    me_w = np.asarray(inputs["me_w"], np.float32)

    bf = ml_dtypes.bfloat16
    xs = x.reshape(NCORES, T, 2, 128, 56, 56)
    xpad = np.zeros((NCORES, T, 2, 128, HP, HP), dtype=bf)
    xpad[:, :, :, :, 1:57, 1:57] = xs.astype(bf)
    # even/odd row planes: [..., 2, 29, 58]
    xeo = np.stack([xpad[:, :, :, :, 0::2, :], xpad[:, :, :, :, 1::2, :]], axis=4)
    xeo = np.ascontiguousarray(xeo.reshape(NCORES, T, 2, 128, PADSZ))

    # G-transformed (over dy) weights; flat [128, 48*128] so the DMA is one
    # contiguous 12.3KB run per partition.  Chunk index:
    # c = co_t*24 + (ci_t*4 + j)*3 + dx, chunk layout [p=ci128, m=co128].
    G = np.array([[1, 0, 0], [.5, .5, .5], [.5, -.5, .5], [0, 0, 1]], np.float32)
    gw_full = np.einsum('jy,oiyx->oijx', G, net_w)       # (256,256,4,3)
    arr = gw_full.reshape(2, 128, 2, 128, 4, 3).transpose(0, 4, 2, 5, 3, 1)
    gw = arr.reshape(48, 128, 128).transpose(1, 0, 2).reshape(128, 6144)
    gw = np.ascontiguousarray(gw.astype(bf))

    gates = _host_gates(x, net_w, net_b, lam_w, lam_b, mlp_w1, bn_g, bn_b,
                        bn_m, bn_v, mlp_w2, me_w)

    in_maps = [dict(x=xeo[i], gw=gw, g=gates[i]) for i in range(NCORES)]
    return in_maps


def kernel(**inputs):
    in_maps = _prep(inputs)
    nc = _CACHE.get('nc')
    if nc is None:
        nc = _build()
        _CACHE['nc'] = nc
    res = run_bass_kernel_spmd(nc, in_maps, core_ids=list(range(NCORES)))
    out = np.stack([res.results[i]["out"] for i in range(NCORES)])  # (8,8,256,3136)
    return np.ascontiguousarray(out.reshape(64, 256, 56, 56))
